# revision 1
# baseline (speedup 1.0000x reference)
"""DeepSeekMoE layer on 8 TRN2 NeuronCores — expert-parallel with host token dispatch.

Reference computation (per token):
    shared = silu(x @ ws1) @ ws2
    router: softmax(x @ w_router) -> top-2 -> renormalize -> gates
    routed = sum_{e in top2} gate_e * silu(x @ w1[e]) @ w2[e]
    out    = shared + routed

Sharding: expert-parallel. Core e receives the (padded) bucket of all token
rows routed to expert e (capacity C), plus a 1/8 slice of all tokens for the
shared expert. Routing (softmax/top-k) and the dispatch/combine permutations
run on the host; all GEMMs + SiLU + gate scaling run on device.

Per-core device kernel (two passes through a DRAM-resident intermediate):
    pass1: hT = silu(w1e.T @ xT)   [I x R] with I=1408 on partitions
    pass2: y  = hT.T @ w2e         [R x H] scaled per-row by the gate
All matmuls run as float32r (full fp32 storage; TensorE full-rate mode).
"""

import numpy as np

import concourse.mybir as mybir
import concourse.tile as tile
from concourse import bacc
from concourse.bass_utils import run_bass_kernel_spmd

H = 2048          # hidden
I = 1408          # moe intermediate
E = 8             # routed experts == n cores
NCORES = 8
RT = 512          # token tile (columns of xT / rows of y) per step
KH = H // 128     # 16 k-tiles over hidden
KI = I // 128     # 11 k-tiles over intermediate
F32 = mybir.dt.float32
F32R = mybir.dt.float32r

_BUILD_CACHE: dict = {}

NPRE = 4          # weight k-slices prefetched into wpre for the next phase
STORE_ENGINE = "sync"   # which engine issues output stores: sync|gpsimd|scalar|vector
IN_BUFS = 4


def _store(nc, dst, src):
    getattr(nc, STORE_ENGINE).dma_start(dst, src)


def _silu_evict(nc, out_pool, ps, tag_id, use_silu, sz=RT):
    ev = out_pool.tile([128, RT], F32R, tag="ev", name=f"ev_{tag_id}")
    if use_silu:
        nc.scalar.activation(ev[:, :sz], ps[:], mybir.ActivationFunctionType.Silu)
    else:
        sg = out_pool.tile([128, RT], F32, tag="sg", bufs=2, name=f"sg_{tag_id}")
        nc.scalar.activation(sg[:, :sz], ps[:], mybir.ActivationFunctionType.Sigmoid)
        nc.vector.tensor_mul(ev[:, :sz], ps[:], sg[:, :sz])
    return ev


def _tiles(ncols):
    """R-tile (offset, size) list: 512-tiles plus an optional 256 tail."""
    out, off = [], 0
    while ncols - off >= RT:
        out.append((off, RT))
        off += RT
    if ncols - off:
        assert (ncols - off) % 256 == 0
        out.append((off, ncols - off))
        off = ncols
    return out


def _emit_pass1(nc, pools, wpool, w_dram, w_pre, xt, ht, ncols, use_silu, ph):
    """ht[:, i, :] = silu(w.T @ xT) — I on partitions, tokens on free.

    First R-tile: per-k sliced stream+weight DMAs in consumption order and a
    k-outer chunked matmul loop, so the PE consumes slices as they arrive.
    Later R-tiles: i-outer / k-inner with whole-half stream DMAs.
    """
    in_pool, out_pool, psum_pool = pools
    npre = len(w_pre) if w_pre else 0

    # --- first R-tile: sliced, streaming ---
    cs = slice(0, RT)
    xh0 = in_pool.tile([128, 8, RT], F32R, tag="sin", name=f"xh0_{ph}_0")
    xh1 = in_pool.tile([128, 8, RT], F32R, tag="sin", name=f"xh1_{ph}_0")
    w = wpool.tile([128, KH, I], F32R, tag="w", name=f"w_{ph}")

    def xslice(k):
        return (xh0 if k < 8 else xh1)[:, k % 8, :]

    if npre:
        # stream slices first (prefetchable), then the WAR-blocked weight rest
        for k in range(KH):
            nc.sync.dma_start(xslice(k), xt[:, k, cs])
        for k in range(npre, KH):
            nc.sync.dma_start(w[:, k, :], w_dram[:, k, :])
    else:
        # cold start: interleave in consumption order
        for k in range(KH):
            nc.sync.dma_start(xslice(k), xt[:, k, cs])
            nc.sync.dma_start(w[:, k, :], w_dram[:, k, :])

    for lo, hi in ((0, 6), (6, KI)):
        pss = [psum_pool.tile([128, RT], F32, tag="ps", name=f"ps_{ph}_0_{i}")
               for i in range(lo, hi)]
        for k in range(KH):
            w_k = w_pre[k] if k < npre else w[:, k, :]
            for i in range(lo, hi):
                nc.tensor.matmul(
                    pss[i - lo][:], w_k[:, i * 128:(i + 1) * 128], xslice(k),
                    start=(k == 0), stop=(k == KH - 1))
        for i in range(lo, hi):
            ev = _silu_evict(nc, out_pool, pss[i - lo], f"{ph}_0_{i}", use_silu)
            _store(nc, ht[:, i, cs], ev[:])
    if npre:
        # also load the wpre-covered slices into the main tile for later R-tiles
        for k in range(npre):
            nc.sync.dma_start(w[:, k, :], w_dram[:, k, :])

    next_pre = None

    # --- remaining R-tiles (maybe a 256-wide tail) ---
    for r, (off, sz) in enumerate(_tiles(ncols)):
        if r == 0:
            continue
        cs = slice(off, off + sz)
        xh0 = in_pool.tile([128, 8, sz], F32R, tag="sin", name=f"xh0_{ph}_{r}")
        xh1 = in_pool.tile([128, 8, sz], F32R, tag="sin", name=f"xh1_{ph}_{r}")
        nc.sync.dma_start(xh0[:], xt[:, 0:8, cs])
        nc.sync.dma_start(xh1[:], xt[:, 8:16, cs])
        for i in range(KI):
            ps = psum_pool.tile([128, sz], F32, tag="ps", name=f"ps_{ph}_{r}_{i}")
            for k in range(KH):
                nc.tensor.matmul(
                    ps[:], w[:, k, i * 128:(i + 1) * 128],
                    (xh0 if k < 8 else xh1)[:, k % 8, :],
                    start=(k == 0), stop=(k == KH - 1))
            ev = _silu_evict(nc, out_pool, ps, f"{ph}_{r}_{i}", use_silu, sz)
            _store(nc, ht[:, i, cs], ev[:, :sz])
        if r == 1:
            next_pre = yield  # build() emits the next phase's wpre DMAs here
    if len(_tiles(ncols)) < 2:
        next_pre = yield
    yield next_pre


def _emit_pass2(nc, pools, wpool, w_dram, w_pre, ht, y, ncols, scale_sb, ph):
    """y[m, :] = (hT.T @ w2) * gate[m] — tokens on partitions.

    First R-tile: per-i sliced DMAs + i-outer over two chunks of 8 psum banks.
    Later R-tiles: i-inner.
    """
    in_pool, out_pool, psum_pool = pools
    npre = len(w_pre) if w_pre else 0

    def evict(ps, m, hblk):
        yt = out_pool.tile([128, 512], F32, tag="ev", name=f"y_{ph}_{m}_{hblk}")
        if scale_sb is not None:
            nc.vector.tensor_scalar_mul(yt[:], ps[:], scale_sb[:, m:m + 1])
        else:
            nc.vector.tensor_copy(yt[:], ps[:])
        _store(nc, y[:, m, hblk * 512:(hblk + 1) * 512], yt[:])

    # --- first R-tile: sliced, streaming ---
    cs = slice(0, RT)
    hh0 = in_pool.tile([128, 6, RT], F32R, tag="sin", name=f"hh0_{ph}_0")
    hh1 = in_pool.tile([128, 5, RT], F32R, tag="sin", name=f"hh1_{ph}_0")
    w = wpool.tile([128, KI, H], F32R, tag="w", name=f"w_{ph}")

    def hslice(i):
        return (hh0 if i < 6 else hh1)[:, i if i < 6 else i - 6, :]

    def lhs(i, c):
        return hslice(i)[:, c * 128:(c + 1) * 128]

    for i in range(KI):
        nc.sync.dma_start(hslice(i), ht[:, i, cs])
    for i in range(npre, KI):
        nc.sync.dma_start(w[:, i, :], w_dram[:, i, :])

    pairs = [(c, hb) for c in range(RT // 128) for hb in range(H // 512)]
    for chunk in (pairs[:8], pairs[8:]):
        pss = {p: psum_pool.tile([128, 512], F32, tag="ps",
                                 name=f"ps_{ph}_0_{p[0]}_{p[1]}")
               for p in chunk}
        for i in range(KI):
            w_i = w_pre[i] if i < npre else w[:, i, :]
            for (c, hb) in chunk:
                nc.tensor.matmul(
                    pss[(c, hb)][:], lhs(i, c), w_i[:, hb * 512:(hb + 1) * 512],
                    start=(i == 0), stop=(i == KI - 1))
        for (c, hb) in chunk:
            evict(pss[(c, hb)], c, hb)
    for i in range(npre):
        nc.sync.dma_start(w[:, i, :], w_dram[:, i, :])

    next_pre = None

    # --- remaining R-tiles (maybe a 256-wide tail) ---
    for r, (off, sz) in enumerate(_tiles(ncols)):
        if r == 0:
            continue
        cs = slice(off, off + sz)
        hh0 = in_pool.tile([128, 6, sz], F32R, tag="sin", name=f"hh0_{ph}_{r}")
        hh1 = in_pool.tile([128, 5, sz], F32R, tag="sin", name=f"hh1_{ph}_{r}")
        nc.sync.dma_start(hh0[:], ht[:, 0:6, cs])
        nc.sync.dma_start(hh1[:], ht[:, 6:KI, cs])
        for c in range(sz // 128):
            for hb in range(H // 512):
                m = off // 128 + c
                ps = psum_pool.tile([128, 512], F32, tag="ps", name=f"ps_{ph}_{m}_{hb}")
                for i in range(KI):
                    src = hh0 if i < 6 else hh1
                    j = i if i < 6 else i - 6
                    nc.tensor.matmul(
                        ps[:], src[:, j, c * 128:(c + 1) * 128],
                        w[:, i, hb * 512:(hb + 1) * 512],
                        start=(i == 0), stop=(i == KI - 1))
                evict(ps, m, hb)
        if r == 1:
            next_pre = yield
    if len(_tiles(ncols)) < 2:
        next_pre = yield
    yield next_pre


def build(C, S, debug=False, use_silu=True, reps=1):
    """Build the per-core Bass module. C: expert capacity, S: shared rows.

    reps>1 repeats the whole computation in one NEFF (timing use only)."""
    assert C % 256 == 0 and C >= RT and S % RT == 0
    nc = bacc.Bacc(None, target_bir_lowering=False, debug=debug)
    with tile.TileContext(nc) as tc:
        with tc.tile_pool(name="dram", bufs=1, space="DRAM") as dram:
            xtd = dram.tile((128, KH, C), F32R, kind="ExternalInput", name="xtd", uniquify=False)
            xts = dram.tile((128, KH, S), F32R, kind="ExternalInput", name="xts", uniquify=False)
            w1e = dram.tile((128, KH, I), F32R, kind="ExternalInput", name="w1e", uniquify=False)
            w2e = dram.tile((128, KI, H), F32R, kind="ExternalInput", name="w2e", uniquify=False)
            ws1 = dram.tile((128, KH, I), F32R, kind="ExternalInput", name="ws1", uniquify=False)
            ws2 = dram.tile((128, KI, H), F32R, kind="ExternalInput", name="ws2", uniquify=False)
            gate = dram.tile((128, C // 128), F32, kind="ExternalInput", name="gate", uniquify=False)
            yd = dram.tile((128, C // 128, H), F32, kind="ExternalOutput", name="yd", uniquify=False)
            ys = dram.tile((128, S // 128, H), F32, kind="ExternalOutput", name="ys", uniquify=False)
            htd = dram.tile((128, KI, C), F32R, name="htd", uniquify=False)
            hts = dram.tile((128, KI, S), F32R, name="hts", uniquify=False)

            with (
                tc.tile_pool(name="wpool", bufs=1) as wpool,
                tc.tile_pool(name="wpre", bufs=NPRE) as wpre_pool,
                tc.tile_pool(name="inpool", bufs=IN_BUFS) as in_pool,
                tc.tile_pool(name="outpool", bufs=6) as out_pool,
                tc.tile_pool(name="psum", bufs=8, space="PSUM") as psum_pool,
                tc.tile_pool(name="const", bufs=1) as const_pool,
            ):
                pools = (in_pool, out_pool, psum_pool)
                scale_sb = const_pool.tile([128, C // 128], F32, name="scale_sb")
                nc.sync.dma_start(scale_sb[:], gate[:])

                def load_pre(dram_w, tag_id):
                    pre = []
                    for k in range(NPRE):
                        t = wpre_pool.tile([128, H], F32R, tag="wpre",
                                           name=f"wpre_{tag_id}_{k}")
                        tv = t[:, :dram_w.shape[2]]
                        nc.sync.dma_start(tv, dram_w[:, k, :])
                        pre.append(tv)
                    return pre

                pre_a = None
                for rep in range(reps):
                    gen = _emit_pass1(nc, pools, wpool, w1e, pre_a, xtd, htd, C,
                                      use_silu, f"a{rep}")
                    next(gen)
                    pre_c = gen.send(load_pre(w2e, f"c{rep}"))

                    gen = _emit_pass2(nc, pools, wpool, w2e, pre_c, htd, yd, C,
                                      scale_sb, f"c{rep}")
                    next(gen)
                    pre_b = gen.send(load_pre(ws1, f"b{rep}"))

                    gen = _emit_pass1(nc, pools, wpool, ws1, pre_b, xts, hts, S,
                                      use_silu, f"b{rep}")
                    next(gen)
                    pre_d = gen.send(load_pre(ws2, f"d{rep}"))

                    gen = _emit_pass2(nc, pools, wpool, ws2, pre_d, hts, ys, S,
                                      None, f"d{rep}")
                    next(gen)
                    pre_a = gen.send(load_pre(w1e, f"a{rep + 1}")
                                     if rep + 1 < reps else None)

    nc.compile()
    return nc


def _get_built(C, S):
    key = (C, S)
    if key not in _BUILD_CACHE:
        _BUILD_CACHE[key] = build(C, S)
    return _BUILD_CACHE[key]


def _to_kxm_layout(a):
    """[K, M] -> [128, K/128, M] with logical row k at (k%128, k//128)."""
    k, m_ = a.shape
    return np.ascontiguousarray(a.reshape(k // 128, 128, m_).transpose(1, 0, 2))


def _round_fp32r(a):
    """Round fp32 to the fp32r grid (RNE to 1s+8e+11m; low 12 bits zero)."""
    u = np.ascontiguousarray(a).view(np.uint32)
    lsb = (u >> 12) & 1
    return ((u + 0x7FF + lsb) & 0xFFFFF000).view(np.float32)


def route_and_dispatch(xf, w_router):
    """Host router: returns (sorted token ids, gates, per-expert offsets, capacity)."""
    T = xf.shape[0]
    logits = xf @ w_router                       # [T, E]
    order = np.argsort(-logits, axis=1, kind="stable")[:, :2]
    mx = logits.max(axis=1, keepdims=True)
    p = np.exp(logits - mx)
    p /= p.sum(axis=1, keepdims=True)
    tk = np.take_along_axis(p, order, axis=1)    # [T, 2]
    g = tk / tk.sum(axis=1, keepdims=True)

    pe = order.ravel()                           # expert id per (token, slot) pair
    ptok = np.repeat(np.arange(T, dtype=np.int64), 2)
    pg = g.astype(np.float32).ravel()
    perm = np.argsort(pe, kind="stable")
    stok, sg = ptok[perm], pg[perm]
    counts = np.bincount(pe, minlength=E)
    offs = np.zeros(E + 1, dtype=np.int64)
    np.cumsum(counts, out=offs[1:])
    C = max(512, int(-(-counts.max() // 256) * 256))
    return stok, sg, offs, C


def prepare(x, w_shared1, w_shared2, w1, w2, w_router):
    """Host-side routing + dispatch. Returns (in_maps, meta)."""
    x = np.asarray(x, dtype=np.float32)
    w_shared1 = np.asarray(w_shared1, dtype=np.float32)
    w_shared2 = np.asarray(w_shared2, dtype=np.float32)
    w1 = np.asarray(w1, dtype=np.float32)
    w2 = np.asarray(w2, dtype=np.float32)
    w_router = np.asarray(w_router, dtype=np.float32)

    B, Sq, _ = x.shape
    T = B * Sq
    S = T // NCORES                              # shared-expert rows per core
    xf = x.reshape(T, H)

    stok, sg, offs, C = route_and_dispatch(xf, w_router)

    # pre-round matmul operands to the fp32r grid (router used unrounded xf)
    xf = _round_fp32r(xf)
    ws1_l = _to_kxm_layout(_round_fp32r(w_shared1))
    ws2_l = _to_kxm_layout(_round_fp32r(w_shared2))
    w1 = _round_fp32r(w1)
    w2 = _round_fp32r(w2)

    in_maps = []
    for e in range(NCORES):
        toks = stok[offs[e]:offs[e + 1]]
        n = len(toks)
        xd = np.zeros((C, H), np.float32)
        xd[:n] = xf[toks]
        gate_v = np.zeros(C, np.float32)
        gate_v[:n] = sg[offs[e]:offs[e + 1]]
        xs = xf[e * S:(e + 1) * S]
        in_maps.append({
            "xtd": np.ascontiguousarray(xd.reshape(C, KH, 128).transpose(2, 1, 0)),
            "xts": np.ascontiguousarray(xs.reshape(S, KH, 128).transpose(2, 1, 0)),
            "w1e": _to_kxm_layout(w1[e]),
            "w2e": _to_kxm_layout(w2[e]),
            "ws1": ws1_l,
            "ws2": ws2_l,
            "gate": np.ascontiguousarray(gate_v.reshape(C // 128, 128).T),
        })

    meta = (B, Sq, T, S, C, stok, offs)
    return in_maps, meta


def combine(results, meta):
    """Host-side gather/unshard of per-core outputs to the full output."""
    B, Sq, T, S, C, stok, offs = meta
    out = np.zeros((T, H), np.float32)
    for e in range(NCORES):
        toks = stok[offs[e]:offs[e + 1]]
        ydp = results[e]["yd"].transpose(1, 0, 2).reshape(C, H)
        out[toks] += ydp[:len(toks)]
        ysp = results[e]["ys"].transpose(1, 0, 2).reshape(S, H)
        out[e * S:(e + 1) * S] += ysp
    return out.reshape(B, Sq, H)


def kernel(x, w_shared1, w_shared2, w1, w2, w_router):
    in_maps, meta = prepare(x, w_shared1, w_shared2, w1, w2, w_router)
    C, S = meta[4], meta[3]
    nc = _get_built(C, S)
    res = run_bass_kernel_spmd(nc, in_maps, core_ids=list(range(NCORES)))
    return combine(res.results, meta)



# revision 2
# speedup vs baseline: 1.3373x; 1.3373x over previous
"""DeepSeekMoE layer on 8 TRN2 NeuronCores — expert-parallel, fused bf16 pipeline.

Reference computation (per token):
    shared = silu(x @ ws1) @ ws2
    router: softmax(x @ w_router) -> top-2 -> renormalize -> gates
    routed = sum_{e in top2} gate_e * silu(x @ w1[e]) @ w2[e]
    out    = shared + routed

Sharding: expert-parallel. Core e gets the (padded to 256) bucket of rows
routed to expert e (capacity C) plus a 1/8 slice of all tokens for the shared
expert (S rows). Routing + dispatch/combine permutations run on the host.

Device kernel (per core): a flat stream of 512-row blocks (routed blocks
using w1e/w2e, then shared blocks using ws1/ws2), each fused:
    p1: h = silu(w1.T @ xT)    [I on partitions, tokens on free]  (psum->SBUF)
    p2: y = h.T @ w2 (* gate)  [tokens on partitions, H on free]
All operands bf16 (same PE rate as fp32r, half the SBUF/DMA); psum fp32.
Both weight matrices of the active phase stay SBUF-resident; blocks are
processed in PAIRS so each stationary weight tile feeds 2 consecutive
matmuls in p1 and 4 in p2 (amortizes PE weight-load time), and emission is
software-pipelined one pair ahead (p1(m+1) before p2(m)) so the PE never
waits on psum evictions.
"""

import numpy as np
import ml_dtypes

import concourse.mybir as mybir
import concourse.tile as tile
from concourse import bacc
from concourse.bass_utils import run_bass_kernel_spmd

H = 2048          # hidden
I = 1408          # moe intermediate
E = 8             # routed experts == n cores
NCORES = 8
KH = H // 128     # 16 k-tiles over hidden
KI = I // 128     # 11 k-tiles over intermediate
NHB = H // 512    # 4 output column chunks in pass2
F32 = mybir.dt.float32
BF16 = mybir.dt.bfloat16
NPBF = ml_dtypes.bfloat16

_BUILD_CACHE: dict = {}
DEDUPE_LW = False


def _lw_sig(lw):
    ap = lw.ins[0]
    return (str(ap.ap), ap.offset, ap.memref, str(ap.dtype),
            str(lw.perf_mode), str(lw.is_transpose),
            str(lw.tile_position), str(lw.tile_size))


def _dedupe_ldweights(nc):
    """Remove InstLdweights that reload the exact weights already resident in
    the PE array (only matmuls in between).  Waits/updates of the removed
    instruction are merged into the following matmul.  Runs pre-compile so
    move_matmul_waits_to_ldweights / generate_event_semaphores legalize the
    result."""
    n_rm = 0
    for blk in nc.m.functions[0].blocks:
        insts = blk.instructions
        last_sig = None
        to_remove = set()
        for idx, ins in enumerate(insts):
            tn = type(ins).__name__
            if tn == "InstLdweights":
                sig = _lw_sig(ins)
                if sig == last_sig and idx + 1 < len(insts) and \
                        type(insts[idx + 1]).__name__ == "InstMatmult":
                    nxt = insts[idx + 1]
                    si = ins.sync_info
                    if si is not None and (si.on_wait or si.on_update):
                        ni = nxt.sync_info
                        if ni is None:
                            nxt.sync_info = mybir.SyncInfo(
                                on_wait=list(si.on_wait), on_update=list(si.on_update))
                        else:
                            ni.on_wait = list(ni.on_wait) + list(si.on_wait)
                            ni.on_update = list(ni.on_update) + list(si.on_update)
                    nxt.merge_dependencies_from(ins)
                    to_remove.add(id(ins))
                else:
                    last_sig = sig
            elif tn == "InstMatmult":
                pass  # matmuls don't clobber the loaded weights
            else:
                last_sig = None  # anything else: conservatively reset
        if to_remove:
            insts[:] = [i for i in insts if id(i) not in to_remove]
            n_rm += len(to_remove)
    return n_rm


def _blocks(C, S):
    """Block list: (src, off, sz, shared)."""
    out = []
    for off in range(0, C, 512):
        out.append(("d", off, min(512, C - off), False))
    for off in range(0, S, 512):
        out.append(("s", off, 512, True))
    return out


def _pairs(blocks):
    """Pair adjacent full blocks with the same weight set; leftovers single."""
    out, i = [], 0
    while i < len(blocks):
        a = blocks[i]
        if (i + 1 < len(blocks) and blocks[i + 1][3] == a[3]
                and a[2] == 512 and blocks[i + 1][2] == 512):
            out.append((a, blocks[i + 1]))
            i += 2
        else:
            out.append((a,))
            i += 1
    return out


def build(C, S, debug=False, reps=1):
    """Build the per-core Bass module. C: expert capacity, S: shared rows.

    reps>1 repeats the whole computation in one NEFF (timing use only)."""
    assert C % 256 == 0 and C >= 512 and S % 512 == 0
    nc = bacc.Bacc(None, target_bir_lowering=False, debug=debug)
    with tile.TileContext(nc) as tc:
        with tc.tile_pool(name="dram", bufs=1, space="DRAM") as dram:
            xtd = dram.tile((128, KH, C), BF16, kind="ExternalInput", name="xtd", uniquify=False)
            xts = dram.tile((128, KH, S), BF16, kind="ExternalInput", name="xts", uniquify=False)
            w1e = dram.tile((128, KH, I), BF16, kind="ExternalInput", name="w1e", uniquify=False)
            w2e = dram.tile((128, KI, H), BF16, kind="ExternalInput", name="w2e", uniquify=False)
            ws1 = dram.tile((128, KH, I), BF16, kind="ExternalInput", name="ws1", uniquify=False)
            ws2 = dram.tile((128, KI, H), BF16, kind="ExternalInput", name="ws2", uniquify=False)
            gate = dram.tile((128, C // 128), F32, kind="ExternalInput", name="gate", uniquify=False)
            yd = dram.tile((128, C // 128, H), BF16, kind="ExternalOutput", name="yd", uniquify=False)
            ys = dram.tile((128, S // 128, H), BF16, kind="ExternalOutput", name="ys", uniquify=False)

            xsrc = {"d": xtd, "s": xts}
            ysrc = {"d": yd, "s": ys}
            w1src = {False: w1e, True: ws1}
            w2src = {False: w2e, True: ws2}

            with (
                tc.tile_pool(name="w1pool", bufs=1) as w1pool,
                tc.tile_pool(name="w2pool", bufs=1) as w2pool,
                tc.tile_pool(name="xpool", bufs=4) as xpool,
                tc.tile_pool(name="hpool", bufs=4) as hpool,
                tc.tile_pool(name="ypool", bufs=8) as ypool,
                tc.tile_pool(name="psum", bufs=8, space="PSUM") as psum_pool,
                tc.tile_pool(name="const", bufs=1) as cpool,
            ):
                scale_sb = cpool.tile([128, C // 128], F32, name="scale_sb")
                nc.sync.dma_start(scale_sb[:], gate[:])

                pair_list = _pairs(_blocks(C, S)) * reps
                M = len(pair_list)

                cur_w1 = [None, None]  # [key, tile]
                cur_w2 = [None, None]

                def ensure_w1(shared, uid):
                    if cur_w1[0] != shared:
                        t = w1pool.tile([128, KH, I], BF16, tag="w1", name=f"w1_{uid}")
                        for k in range(KH):
                            nc.sync.dma_start(t[:, k, :], w1src[shared][:, k, :])
                        cur_w1[0], cur_w1[1] = shared, t
                    return cur_w1[1]

                def ensure_w2(shared, uid):
                    if cur_w2[0] != shared:
                        t = w2pool.tile([128, KI, H], BF16, tag="w2", name=f"w2_{uid}")
                        for i in range(KI):
                            nc.sync.dma_start(t[:, i, :], w2src[shared][:, i, :])
                        cur_w2[0], cur_w2[1] = shared, t
                    return cur_w2[1]

                def load_x(pr, uid):
                    ts = []
                    for b, (src, off, sz, _sh) in enumerate(pr):
                        t = xpool.tile([128, KH, 512], BF16, tag="x", name=f"x_{uid}_{b}")
                        nc.sync.dma_start(t[:, 0:8, :sz], xsrc[src][:, 0:8, off:off + sz])
                        nc.sync.dma_start(t[:, 8:KH, :sz], xsrc[src][:, 8:KH, off:off + sz])
                        ts.append(t)
                    return ts

                def emit_p1(pr, w1t, xts_, uid):
                    """h(b) = silu(w1.T @ x(b)); returns h tiles."""
                    nb = len(pr)
                    hts_ = [hpool.tile([128, KI, 512], BF16, tag="h", name=f"h_{uid}_{b}")
                            for b in range(nb)]
                    if nb == 2:
                        ics = [(0, 2), (2, 4), (4, 6), (6, 8), (8, 10), (10, 11)]
                    else:
                        ics = [(0, 4), (4, 8), (8, 11)]
                    for lo, hi in ics:
                        pss = {}
                        for i in range(lo, hi):
                            for b in range(nb):
                                pss[(i, b)] = psum_pool.tile(
                                    [128, pr[b][2]], F32, tag="ps", name=f"ps1_{uid}_{i}_{b}")
                        for k in range(KH):
                            for i in range(lo, hi):
                                wk = w1t[:, k, i * 128:(i + 1) * 128]
                                for b in range(nb):
                                    nc.tensor.matmul(
                                        pss[(i, b)][:], wk, xts_[b][:, k, :pr[b][2]],
                                        start=(k == 0), stop=(k == KH - 1))
                        for i in range(lo, hi):
                            for b in range(nb):
                                nc.scalar.activation(
                                    hts_[b][:, i, :pr[b][2]], pss[(i, b)][:],
                                    mybir.ActivationFunctionType.Silu)
                    return hts_

                def emit_p2(pr, w2t, hts_, uid):
                    """y(b) = (h(b).T @ w2) * gate, streamed out per 128-row chunk."""
                    for b, (src, off, sz, shared) in enumerate(pr):
                        for c in range(sz // 128):
                            m = off // 128 + c
                            pss = [psum_pool.tile([128, 512], F32, tag="ps",
                                                  name=f"ps2_{uid}_{b}_{c}_{hb}")
                                   for hb in range(NHB)]
                            for i in range(KI):
                                stat = hts_[b][:, i, c * 128:(c + 1) * 128]
                                for hb in range(NHB):
                                    nc.tensor.matmul(
                                        pss[hb][:], stat, w2t[:, i, hb * 512:(hb + 1) * 512],
                                        start=(i == 0), stop=(i == KI - 1))
                            for hb in range(NHB):
                                yt = ypool.tile([128, 512], BF16, tag="y",
                                                name=f"y_{uid}_{b}_{c}_{hb}")
                                if shared:
                                    nc.vector.tensor_copy(yt[:], pss[hb][:])
                                else:
                                    nc.vector.tensor_scalar_mul(
                                        yt[:], pss[hb][:], scale_sb[:, m:m + 1])
                                nc.gpsimd.dma_start(
                                    ysrc[src][:, m, hb * 512:(hb + 1) * 512], yt[:])

                # prologue: x for pairs 0 and 1
                xts_q = [load_x(pair_list[0], "p0")]
                if M > 1:
                    xts_q.append(load_x(pair_list[1], "p1"))
                h_q = []

                for m in range(M):
                    pr = pair_list[m]
                    w1t = ensure_w1(pr[0][3], f"m{m}")
                    h_q.append(emit_p1(pr, w1t, xts_q[0], f"m{m}"))
                    xts_q.pop(0)
                    if m + 2 < M:
                        xts_q.append(load_x(pair_list[m + 2], f"p{m + 2}"))
                    if m >= 1:
                        prv = pair_list[m - 1]
                        w2t = ensure_w2(prv[0][3], f"m{m - 1}")
                        emit_p2(prv, w2t, h_q.pop(0), f"m{m - 1}")
                # epilogue
                prv = pair_list[M - 1]
                w2t = ensure_w2(prv[0][3], f"m{M - 1}")
                emit_p2(prv, w2t, h_q.pop(0), f"m{M - 1}")

    if DEDUPE_LW:
        n = _dedupe_ldweights(nc)
        if debug:
            print(f"deduped {n} InstLdweights")
    nc.compile()
    return nc


def _get_built(C, S):
    key = (C, S)
    if key not in _BUILD_CACHE:
        _BUILD_CACHE[key] = build(C, S)
    return _BUILD_CACHE[key]


def _to_kxm_layout(a):
    """[K, M] -> [128, K/128, M] with logical row k at (k%128, k//128)."""
    k, m_ = a.shape
    return np.ascontiguousarray(a.reshape(k // 128, 128, m_).transpose(1, 0, 2))


def route_and_dispatch(xf, w_router):
    """Host router: returns (sorted token ids, gates, per-expert offsets, capacity)."""
    T = xf.shape[0]
    logits = xf @ w_router                       # [T, E]
    order = np.argsort(-logits, axis=1, kind="stable")[:, :2]
    mx = logits.max(axis=1, keepdims=True)
    p = np.exp(logits - mx)
    p /= p.sum(axis=1, keepdims=True)
    tk = np.take_along_axis(p, order, axis=1)    # [T, 2]
    g = tk / tk.sum(axis=1, keepdims=True)

    pe = order.ravel()                           # expert id per (token, slot) pair
    ptok = np.repeat(np.arange(T, dtype=np.int64), 2)
    pg = g.astype(np.float32).ravel()
    perm = np.argsort(pe, kind="stable")
    stok, sg = ptok[perm], pg[perm]
    counts = np.bincount(pe, minlength=E)
    offs = np.zeros(E + 1, dtype=np.int64)
    np.cumsum(counts, out=offs[1:])
    C = max(512, int(-(-counts.max() // 256) * 256))
    return stok, sg, offs, C


def prepare(x, w_shared1, w_shared2, w1, w2, w_router):
    """Host-side routing + dispatch. Returns (in_maps, meta)."""
    x = np.asarray(x, dtype=np.float32)
    w_shared1 = np.asarray(w_shared1, dtype=np.float32)
    w_shared2 = np.asarray(w_shared2, dtype=np.float32)
    w1 = np.asarray(w1, dtype=np.float32)
    w2 = np.asarray(w2, dtype=np.float32)
    w_router = np.asarray(w_router, dtype=np.float32)

    B, Sq, _ = x.shape
    T = B * Sq
    S = T // NCORES                              # shared-expert rows per core
    xf = x.reshape(T, H)

    stok, sg, offs, C = route_and_dispatch(xf, w_router)

    xb = xf.astype(NPBF)
    ws1_l = _to_kxm_layout(w_shared1.astype(NPBF))
    ws2_l = _to_kxm_layout(w_shared2.astype(NPBF))
    w1b = w1.astype(NPBF)
    w2b = w2.astype(NPBF)

    in_maps = []
    for e in range(NCORES):
        toks = stok[offs[e]:offs[e + 1]]
        n = len(toks)
        xd = np.zeros((C, H), NPBF)
        xd[:n] = xb[toks]
        gate_v = np.zeros(C, np.float32)
        gate_v[:n] = sg[offs[e]:offs[e + 1]]
        xs = xb[e * S:(e + 1) * S]
        in_maps.append({
            "xtd": np.ascontiguousarray(xd.reshape(C, KH, 128).transpose(2, 1, 0)),
            "xts": np.ascontiguousarray(xs.reshape(S, KH, 128).transpose(2, 1, 0)),
            "w1e": _to_kxm_layout(w1b[e]),
            "w2e": _to_kxm_layout(w2b[e]),
            "ws1": ws1_l,
            "ws2": ws2_l,
            "gate": np.ascontiguousarray(gate_v.reshape(C // 128, 128).T),
        })

    meta = (B, Sq, T, S, C, stok, offs)
    return in_maps, meta


def combine(results, meta):
    """Host-side gather/unshard of per-core outputs to the full output."""
    B, Sq, T, S, C, stok, offs = meta
    out = np.zeros((T, H), np.float32)
    for e in range(NCORES):
        toks = stok[offs[e]:offs[e + 1]]
        ydp = np.asarray(results[e]["yd"], dtype=np.float32).transpose(1, 0, 2).reshape(C, H)
        out[toks] += ydp[:len(toks)]
        ysp = np.asarray(results[e]["ys"], dtype=np.float32).transpose(1, 0, 2).reshape(S, H)
        out[e * S:(e + 1) * S] += ysp
    return out.reshape(B, Sq, H)


def kernel(x, w_shared1, w_shared2, w1, w2, w_router):
    in_maps, meta = prepare(x, w_shared1, w_shared2, w1, w2, w_router)
    C, S = meta[4], meta[3]
    nc = _get_built(C, S)
    res = run_bass_kernel_spmd(nc, in_maps, core_ids=list(range(NCORES)))
    return combine(res.results, meta)
